# revision 10
# baseline (speedup 1.0000x reference)
"""GroupedTernaryLinear Trainium2 kernel (Bass/Tile, 8-core SPMD).

Computation (matches the jax reference):
  x:      [2, 4096, 4096] f32   -> flatten to [8192, 4096] tokens
  weight: [4096, 1024]    f32
  1. xn = rms_norm(x) over last dim (eps = f32 eps)
  2. w_bf = bf16(weight); per flat 64-chunk: scale = bf16(mean|w_bf|) (clipped),
     q = clip(round(w_bf/scale), -1, 1)  ->  wq = q*scale  (exact in bf16)
  3. out[t, g*1024+o] = sum_i xn[t, g*1024+i] * wq[g*1024+o, i]   (4 groups)

Sharding: 2 token-halves x 4 groups = 8 cores. Core c = 4*i + j gets
tokens [4096*i, 4096*(i+1)) and group j. Each core quantizes only ITS
group's weight. The rms-norm needs the full 4096-feature sum of squares
per token, so the 4 group-shards of a token half run a 2-round
recursive-doubling exchange (XOR 1, XOR 2) of their 16KB per-token
partial sums over remote SBUF-to-SBUF DMA; the norm factor is folded
into the output evacuation.

Key perf choices:
  - NO PE transposes: x and wq are transposed by the DMA XBAR
    (dma_start_transpose, bf16) straight into SBUF, so the tensor engine
    runs nothing but 512-col bf16 matmuls back to back.
  - Matmul runs in two o-half passes (cols 0:512 using weight tiles 0-3,
    then 512:1024 using tiles 4-7) so the first pass starts as soon as
    half the weight is quantized.
  - Pass-0 outputs are staged bf16 and scaled by 1/rms once the factor
    arrives (~70us); pass-1 outputs are mostly scaled straight out of
    PSUM.
"""

import os
import sys

sys.path.insert(0, "/opt/trn_rl_repo")

import numpy as np

import concourse.bass as bass
import concourse.mybir as mybir
import concourse.tile as tile
from concourse import bacc
from concourse.bass_utils import run_bass_kernel_spmd
from concourse.masks import make_identity

F32 = mybir.dt.float32
BF16 = mybir.dt.bfloat16
AF = mybir.ActivationFunctionType
ALU = mybir.AluOpType

N_CORES = 8
TOK = 4096        # tokens per core
DIN = 1024        # per-core input features (one group)
DOUT = 1024       # per-core outputs (one group)
DFULL = 4096      # full feature dim (norm denominator)
TB = TOK // 128   # 32 token blocks
NT = DOUT // 128  # 8 weight tiles of [128 o, 1024 i]
GK = DIN // 128   # 8 k-chunks of 128
EPS = 1.1920929e-07          # np.finfo(np.float32).eps
THR = 0.5009765625           # bf16 round-to-nearest-even threshold for |r|>0.5
STAGE_H1 = 2      # pass-1 blocks staged before the rms factor is ready
MM_LAG = 2        # pass-0 matmul emission lag behind the x pipeline

LAST_EXEC_NS = None
LAST_RESULTS = None


def _build():
    nc = bacc.Bacc("TRN2", target_bir_lowering=False, debug=False, num_devices=8)
    x_ap = nc.dram_tensor("x", [TOK, DIN], F32, kind="ExternalInput").ap()
    w_ap = nc.dram_tensor("weight", [DOUT, DIN], F32, kind="ExternalInput").ap()
    out_ap = nc.dram_tensor("out", [TOK, DOUT], F32, kind="ExternalOutput").ap()

    with tile.TileContext(nc) as tc:
        _body(tc, nc, out_ap, x_ap, w_ap)

    nc.compile()
    return nc


def _body(tc, nc, out_ap, x_ap, w_ap):
    with (
        tc.tile_pool(name="consts", bufs=1) as consts,
        tc.tile_pool(name="wqt", bufs=1) as wqt_pool,
        tc.tile_pool(name="xta", bufs=1) as xta_pool,
        tc.tile_pool(name="rawp", bufs=1) as raw_pool,
        tc.tile_pool(name="win", bufs=4) as win_pool,
        tc.tile_pool(name="wtmp", bufs=2) as wtmp_pool,
        tc.tile_pool(name="wst", bufs=2) as wst_pool,
        tc.tile_pool(name="xin", bufs=5) as xin_pool,
        tc.tile_pool(name="xbfp", bufs=3) as xbf_pool,
        tc.tile_pool(name="stats", bufs=1) as stats_pool,
        tc.tile_pool(name="obp", bufs=3) as ob_pool,
        tc.tile_pool(name="dram", bufs=1, space="DRAM") as dram_pool,
        tc.tile_pool(name="ps_mm", bufs=4, space="PSUM") as ps_mm,
        tc.tile_pool(name="ps_xtp", bufs=2, space="PSUM") as ps_xtp,
        tc.tile_pool(name="ps_wtp", bufs=2, space="PSUM") as ps_wtp,
    ):
        eps_t = consts.tile([128, 1], F32, name="eps_t")
        nc.vector.memset(eps_t[:], EPS)
        ident_b = consts.tile([128, 128], BF16, name="ident_b")
        make_identity(nc, ident_b[:])

        # Quantized transposed weight, split so pass-0 readers never alias
        # the tiles still being written for pass 1: [i(128), k, o-half]
        wqT_lo = wqt_pool.tile([128, GK, 512], BF16, name="wqT_lo")
        wqT_hi = wqt_pool.tile([128, GK, 512], BF16, name="wqT_hi")
        # Per-block transposed x tiles (separate tiles -> exact deps)
        xT = [
            xta_pool.tile([128, GK, 128], BF16, name=f"xT{b}")
            for b in range(TB)
        ]
        # Raw (un-normalized) matmul outputs staged until the factor arrives
        rh0 = [
            raw_pool.tile([128, 512], BF16, name=f"rh0_{b}")
            for b in range(TB)
        ]
        rh1 = [
            raw_pool.tile([128, 512], BF16, name=f"rh1_{b}")
            for b in range(STAGE_H1)
        ]

        ss_all = stats_pool.tile([128, TB], F32, name="ss_all")
        ss_sum = stats_pool.tile([128, TB], F32, name="ss_sum")
        sq_all = stats_pool.tile([128, TB], F32, name="sq_all")
        fac_all = stats_pool.tile([128, TB], F32, name="fac_all")
        junk = stats_pool.tile([128, DIN], BF16, name="junk")

        cc_in = dram_pool.tile([128, TB], F32, name="cc_in")
        cc_out = dram_pool.tile([128, TB], F32, name="cc_out")

        w_tiles = {}

        def emit_wdma(t):
            w_t = win_pool.tile([128, DIN], F32, name="w_t")
            nc.sync.dma_start(w_t[:], w_ap[t * 128:(t + 1) * 128, :])
            w_tiles[t] = w_t

        def emit_wquant(t):
            w_t = w_tiles.pop(t)
            wbf = wtmp_pool.tile([128, DIN], BF16, name="wbf")
            nc.scalar.copy(wbf[:], w_t[:])              # f32 -> bf16 (RNE)
            wbf_v = wbf[:].rearrange("p (c q) -> p c q", q=64)

            red = wst_pool.tile([128, 16], F32, name="red")
            nc.vector.tensor_reduce(
                red[:], wbf_v, axis=mybir.AxisListType.X, op=ALU.add,
                apply_absolute_value=True,
            )
            s_bf = wst_pool.tile([128, 16], BF16, name="s_bf")
            nc.vector.tensor_scalar(
                s_bf[:], red[:], 1.0 / 64.0, 1e-8, ALU.mult, ALU.max,
            )
            thr_p = wst_pool.tile([128, 16], F32, name="thr_p")
            nc.vector.tensor_scalar_mul(thr_p[:], s_bf[:], THR)
            thr_n = wst_pool.tile([128, 16], F32, name="thr_n")
            nc.vector.tensor_scalar_mul(thr_n[:], s_bf[:], -THR)

            # q = (w > t) - (w < -t); wq = q*s  (broadcast views)
            tp_b = thr_p[:].unsqueeze(2).broadcast_to((128, 16, 64))
            tn_b = thr_n[:].unsqueeze(2).broadcast_to((128, 16, 64))
            s_b = s_bf[:].unsqueeze(2).broadcast_to((128, 16, 64))
            mp = wtmp_pool.tile([128, DIN], BF16, name="mp")
            mp_v = mp[:].rearrange("p (c q) -> p c q", q=64)
            nc.vector.tensor_tensor(mp_v, wbf_v, tp_b, ALU.is_gt)
            mn = wtmp_pool.tile([128, DIN], BF16, name="mn")
            mn_v = mn[:].rearrange("p (c q) -> p c q", q=64)
            nc.vector.tensor_tensor(mn_v, wbf_v, tn_b, ALU.is_lt)
            nc.vector.tensor_sub(mp[:], mp[:], mn[:])
            wqv = wtmp_pool.tile([128, DIN], BF16, name="wqv")
            wqv_v = wqv[:].rearrange("p (c q) -> p c q", q=64)
            nc.vector.tensor_tensor(wqv_v, mp_v, s_b, ALU.mult)

            # PE transpose into the resident weight tile:
            # wqT[i, k, (t%4)*128+o] = wqv[o, k*128+i]
            dst = wqT_lo if t < 4 else wqT_hi
            off = (t % 4) * 128
            wps = ps_wtp.tile([128, GK, 128], BF16, name="wps")
            for k in range(GK):
                nc.tensor.transpose(
                    wps[:, k, :], wqv[:, k * 128:(k + 1) * 128], ident_b[:],
                )
            nc.scalar.copy(dst[:, :, off:off + 128], wps[:])

        def emit_xblock(b):
            if b + 5 < TB:
                xn_t = xin_pool.tile([128, DIN], F32, name="x_t")
                nc.sync.dma_start(
                    xn_t[:], x_ap[(b + 5) * 128:(b + 6) * 128, :],
                )
                x_pending.append(xn_t)
            x_t = x_pending.pop(0)   # tile for block b (issued 5 iters ago)
            xb = xbf_pool.tile([128, DIN], BF16, name="xb")
            nc.vector.tensor_copy(xb[:], x_t[:])
            nc.scalar.activation(
                junk[:], xb[:], AF.Square, accum_out=ss_all[:, b:b + 1],
            )
            xps = ps_xtp.tile([128, GK, 128], BF16, name="xps")
            for k in range(GK):
                nc.tensor.transpose(
                    xps[:, k, :], xb[:, k * 128:(k + 1) * 128], ident_b[:],
                )
            nc.vector.tensor_copy(xT[b][:], xps[:])

        def emit_mm_h0(b):
            pm = ps_mm.tile([128, 512], F32, name="pm")
            for k in range(GK):
                nc.tensor.matmul(
                    pm[:], xT[b][:, k, :], wqT_lo[:, k, :],
                    start=(k == 0), stop=(k == GK - 1),
                )
            nc.vector.tensor_copy(rh0[b][:], pm[:])

        def emit_mm_h1(b):
            pm = ps_mm.tile([128, 512], F32, name="pm")
            for k in range(GK):
                nc.tensor.matmul(
                    pm[:], xT[b][:, k, :], wqT_hi[:, k, :],
                    start=(k == 0), stop=(k == GK - 1),
                )
            ob = ob_pool.tile([128, DOUT], F32, name="ob")
            if b < STAGE_H1:
                nc.vector.tensor_copy(rh1[b][:], pm[:])
                nc.scalar.activation(
                    ob[:, 512:1024], rh1[b][:], AF.Copy,
                    scale=fac_all[:, b:b + 1],
                )
            else:
                nc.scalar.activation(
                    ob[:, 512:1024], pm[:], AF.Copy, scale=fac_all[:, b:b + 1],
                )
            nc.scalar.activation(
                ob[:, 0:512], rh0[b][:], AF.Copy, scale=fac_all[:, b:b + 1],
            )
            nc.gpsimd.dma_start(out_ap[b * 128:(b + 1) * 128, :], ob[:])

        # ---- emission ------------------------------------------------------
        x_pending = []
        for t in range(4):
            emit_wdma(t)
        for b in range(5):   # pre-issue x DMA lookahead
            x_t = xin_pool.tile([128, DIN], F32, name="x_t")
            nc.sync.dma_start(x_t[:], x_ap[b * 128:(b + 1) * 128, :])
            x_pending.append(x_t)
        for t in range(4):
            emit_wquant(t)

        w_insert = {6: 4, 12: 5, 18: 6, 24: 7}
        for b in range(TB):
            emit_xblock(b)
            if b in w_insert:
                t = w_insert[b]
                emit_wdma(t)
                emit_wquant(t)
            if b >= MM_LAG:
                emit_mm_h0(b - MM_LAG)
        for b in range(TB - MM_LAG, TB):
            emit_mm_h0(b)

        # ---- cross-core sum of squares (4 group-shards of a token half) ---
        nc.sync.dma_start(cc_in[:], ss_all[:])
        nc.gpsimd.collective_compute(
            "AllReduce",
            ALU.add,
            replica_groups=[[0, 1, 2, 3], [4, 5, 6, 7]],
            ins=[cc_in.opt()],
            outs=[cc_out.opt()],
        )
        nc.sync.dma_start(ss_sum[:], cc_out[:])
        nc.scalar.activation(
            sq_all[:], ss_sum[:], AF.Sqrt, bias=eps_t[:], scale=1.0 / DFULL,
        )
        nc.vector.reciprocal(fac_all[:], sq_all[:])

        # ---- pass 1 + finales ---------------------------------------------
        for b in range(TB):
            emit_mm_h1(b)


_NC_CACHE = None


def _ensure_ntff_hook():
    """Install the antenv.axon_hooks shim + ctypes NTFF hook if missing."""
    import types

    try:
        from antenv.axon_hooks import get_axon_ntff_profile_hook  # noqa: F401
        return
    except ImportError:
        pass
    import antenv

    mod = types.ModuleType("antenv.axon_hooks")
    mod._hook = None
    mod.set_axon_ntff_profile_hook = lambda h: setattr(mod, "_hook", h)
    mod.get_axon_ntff_profile_hook = lambda: mod._hook
    sys.modules["antenv.axon_hooks"] = mod
    antenv.axon_hooks = mod
    try:
        if "/root/.axon_site" not in sys.path:
            sys.path.insert(0, "/root/.axon_site")
        from trn_agent_boot.trn_boot import _ntff_profile_via_ctypes

        mod.set_axon_ntff_profile_hook(
            _ntff_profile_via_ctypes("/opt/axon/libaxon_pjrt.so")
        )
    except Exception:
        pass


def kernel(x: np.ndarray, weight: np.ndarray) -> np.ndarray:
    global LAST_EXEC_NS, LAST_RESULTS, _NC_CACHE
    x = np.ascontiguousarray(np.asarray(x, dtype=np.float32))
    weight = np.ascontiguousarray(np.asarray(weight, dtype=np.float32))
    lead = x.shape[:-1]
    xf = x.reshape(-1, DFULL)
    assert xf.shape[0] == 2 * TOK, xf.shape

    if _NC_CACHE is None:
        _NC_CACHE = _build()
    nc = _NC_CACHE

    in_maps = []
    for i in range(2):
        for j in range(4):
            in_maps.append({
                "x": np.ascontiguousarray(
                    xf[i * TOK:(i + 1) * TOK, j * DIN:(j + 1) * DIN]
                ),
                "weight": np.ascontiguousarray(
                    weight[j * DOUT:(j + 1) * DOUT, :]
                ),
            })
    trace = bool(int(os.environ.get("CCK_TRACE", "0")))
    kw = {}
    if trace:
        _ensure_ntff_hook()
        tdir = os.environ.get("CCK_TRACE_DIR")
        if tdir:
            os.makedirs(tdir, exist_ok=True)
            kw["tmpdir"] = tdir
    res = run_bass_kernel_spmd(nc, in_maps, list(range(N_CORES)), trace=trace, **kw)
    LAST_EXEC_NS = res.exec_time_ns
    LAST_RESULTS = res
    out = np.empty((2 * TOK, DFULL), dtype=np.float32)
    for i in range(2):
        for j in range(4):
            out[i * TOK:(i + 1) * TOK, j * DOUT:(j + 1) * DOUT] = (
                res.results[i * 4 + j]["out"]
            )
    return out.reshape(*lead, DFULL)


if __name__ == "__main__":
    rng = np.random.default_rng(0)
    x = rng.standard_normal((2, 4096, 4096), dtype=np.float32)
    w = (rng.standard_normal((4096, 1024), dtype=np.float32) * 0.02).astype(np.float32)
    o = kernel(x, w)
    print(o.shape, o.dtype, LAST_EXEC_NS)


# revision 11
# speedup vs baseline: 1.0888x; 1.0888x over previous
"""GroupedTernaryLinear Trainium2 kernel (Bass/Tile, 8-core SPMD).

Computation (matches the jax reference):
  x:      [2, 4096, 4096] f32   -> flatten to [8192, 4096] tokens
  weight: [4096, 1024]    f32
  1. xn = rms_norm(x) over last dim (eps = f32 eps)
  2. w_bf = bf16(weight); per flat 64-chunk: scale = bf16(mean|w_bf|) (clipped),
     q = clip(round(w_bf/scale), -1, 1)  ->  wq = q*scale  (exact in bf16)
  3. out[t, g*1024+o] = sum_i xn[t, g*1024+i] * wq[g*1024+o, i]   (4 groups)

Sharding: 2 token-halves x 4 groups = 8 cores. Core c = 4*i + j gets
tokens [4096*i, 4096*(i+1)) and group j. Each core quantizes only ITS
group's weight. The rms-norm needs the full 4096-feature sum of squares
per token, so the 4 group-shards of a token half run a 2-round
recursive-doubling exchange (XOR 1, XOR 2) of their 16KB per-token
partial sums over remote SBUF-to-SBUF DMA; the norm factor is folded
into the output evacuation.

Key perf choices:
  - NO PE transposes: x and wq are transposed by the DMA XBAR
    (dma_start_transpose, bf16) straight into SBUF, so the tensor engine
    runs nothing but 512-col bf16 matmuls back to back.
  - Matmul runs in two o-half passes (cols 0:512 using weight tiles 0-3,
    then 512:1024 using tiles 4-7) so the first pass starts as soon as
    half the weight is quantized.
  - Pass-0 outputs are staged bf16 and scaled by 1/rms once the factor
    arrives (~70us); pass-1 outputs are mostly scaled straight out of
    PSUM.
"""

import os
import sys

sys.path.insert(0, "/opt/trn_rl_repo")

import numpy as np

import concourse.bass as bass
import concourse.mybir as mybir
import concourse.tile as tile
from concourse import bacc
from concourse.bass_utils import run_bass_kernel_spmd
from concourse.masks import make_identity

F32 = mybir.dt.float32
BF16 = mybir.dt.bfloat16
AF = mybir.ActivationFunctionType
ALU = mybir.AluOpType

N_CORES = 8
TOK = 4096        # tokens per core
DIN = 1024        # per-core input features (one group)
DOUT = 1024       # per-core outputs (one group)
DFULL = 4096      # full feature dim (norm denominator)
TB = TOK // 128   # 32 token blocks
NT = DOUT // 128  # 8 weight tiles of [128 o, 1024 i]
GK = DIN // 128   # 8 k-chunks of 128
EPS = 1.1920929e-07          # np.finfo(np.float32).eps
THR = 0.5009765625           # bf16 round-to-nearest-even threshold for |r|>0.5
STAGE_H1 = 2      # pass-1 blocks staged before the rms factor is ready
MM_LAG = 2        # pass-0 matmul emission lag behind the x pipeline

LAST_EXEC_NS = None
LAST_RESULTS = None


def _build():
    nc = bacc.Bacc("TRN2", target_bir_lowering=False, debug=False, num_devices=8)
    x_ap = nc.dram_tensor("x", [TOK, DIN], F32, kind="ExternalInput").ap()
    w_ap = nc.dram_tensor("weight", [DOUT, DIN], F32, kind="ExternalInput").ap()
    out_ap = nc.dram_tensor("out", [TOK, DOUT], F32, kind="ExternalOutput").ap()

    with tile.TileContext(nc) as tc:
        _body(tc, nc, out_ap, x_ap, w_ap)

    nc.compile()
    return nc


def _body(tc, nc, out_ap, x_ap, w_ap):
    with (
        tc.tile_pool(name="consts", bufs=1) as consts,
        tc.tile_pool(name="wqt", bufs=1) as wqt_pool,
        tc.tile_pool(name="xta", bufs=1) as xta_pool,
        tc.tile_pool(name="rawp", bufs=1) as raw_pool,
        tc.tile_pool(name="win", bufs=4) as win_pool,
        tc.tile_pool(name="wtmp", bufs=2) as wtmp_pool,
        tc.tile_pool(name="wst", bufs=2) as wst_pool,
        tc.tile_pool(name="xin", bufs=5) as xin_pool,
        tc.tile_pool(name="xbfp", bufs=3) as xbf_pool,
        tc.tile_pool(name="stats", bufs=1) as stats_pool,
        tc.tile_pool(name="obp", bufs=3) as ob_pool,
        tc.tile_pool(name="dram", bufs=1, space="DRAM") as dram_pool,
        tc.tile_pool(name="ps_mm", bufs=4, space="PSUM") as ps_mm,
        tc.tile_pool(name="ps_xtp", bufs=2, space="PSUM") as ps_xtp,
        tc.tile_pool(name="ps_wtp", bufs=2, space="PSUM") as ps_wtp,
    ):
        eps_t = consts.tile([128, 1], F32, name="eps_t")
        nc.vector.memset(eps_t[:], EPS)
        ident_b = consts.tile([128, 128], BF16, name="ident_b")
        make_identity(nc, ident_b[:])

        # Quantized transposed weight, split so pass-0 readers never alias
        # the tiles still being written for pass 1: [i(128), k, o-half]
        wqT_lo = wqt_pool.tile([128, GK, 512], BF16, name="wqT_lo")
        wqT_hi = wqt_pool.tile([128, GK, 512], BF16, name="wqT_hi")
        # Per-block transposed x tiles (separate tiles -> exact deps)
        xT = [
            xta_pool.tile([128, GK, 128], BF16, name=f"xT{b}")
            for b in range(TB)
        ]
        # Raw (un-normalized) matmul outputs staged until the factor arrives
        rh0 = [
            raw_pool.tile([128, 512], BF16, name=f"rh0_{b}")
            for b in range(TB)
        ]
        rh1 = [
            raw_pool.tile([128, 512], BF16, name=f"rh1_{b}")
            for b in range(STAGE_H1)
        ]

        ss_all = stats_pool.tile([128, TB], F32, name="ss_all")
        ss_sum = stats_pool.tile([128, TB], F32, name="ss_sum")
        sq_all = stats_pool.tile([128, TB], F32, name="sq_all")
        fac_all = stats_pool.tile([128, TB], F32, name="fac_all")
        junk = stats_pool.tile([128, DIN], BF16, name="junk")

        cc_in = dram_pool.tile([128, TB], F32, name="cc_in")
        cc_out = dram_pool.tile([128, TB], F32, name="cc_out")

        w_tiles = {}

        def emit_wdma(t):
            w_t = win_pool.tile([128, DIN], F32, name="w_t")
            nc.sync.dma_start(w_t[:], w_ap[t * 128:(t + 1) * 128, :])
            w_tiles[t] = w_t

        def emit_wquant(t):
            w_t = w_tiles.pop(t)
            wbf = wtmp_pool.tile([128, DIN], BF16, name="wbf")
            nc.scalar.copy(wbf[:], w_t[:])              # f32 -> bf16 (RNE)
            wbf_v = wbf[:].rearrange("p (c q) -> p c q", q=64)

            red = wst_pool.tile([128, 16], F32, name="red")
            nc.vector.tensor_reduce(
                red[:], wbf_v, axis=mybir.AxisListType.X, op=ALU.add,
                apply_absolute_value=True,
            )
            s_bf = wst_pool.tile([128, 16], BF16, name="s_bf")
            nc.vector.tensor_scalar(
                s_bf[:], red[:], 1.0 / 64.0, 1e-8, ALU.mult, ALU.max,
            )
            thr_p = wst_pool.tile([128, 16], F32, name="thr_p")
            nc.vector.tensor_scalar_mul(thr_p[:], s_bf[:], THR)
            thr_n = wst_pool.tile([128, 16], F32, name="thr_n")
            nc.vector.tensor_scalar_mul(thr_n[:], s_bf[:], -THR)

            # q = (w > t) - (w < -t); wq = q*s  (broadcast views)
            tp_b = thr_p[:].unsqueeze(2).broadcast_to((128, 16, 64))
            tn_b = thr_n[:].unsqueeze(2).broadcast_to((128, 16, 64))
            s_b = s_bf[:].unsqueeze(2).broadcast_to((128, 16, 64))
            mp = wtmp_pool.tile([128, DIN], BF16, name="mp")
            mp_v = mp[:].rearrange("p (c q) -> p c q", q=64)
            nc.vector.tensor_tensor(mp_v, wbf_v, tp_b, ALU.is_gt)
            mn = wtmp_pool.tile([128, DIN], BF16, name="mn")
            mn_v = mn[:].rearrange("p (c q) -> p c q", q=64)
            nc.vector.tensor_tensor(mn_v, wbf_v, tn_b, ALU.is_lt)
            nc.vector.tensor_sub(mp[:], mp[:], mn[:])
            wqv = wtmp_pool.tile([128, DIN], BF16, name="wqv")
            wqv_v = wqv[:].rearrange("p (c q) -> p c q", q=64)
            nc.vector.tensor_tensor(wqv_v, mp_v, s_b, ALU.mult)

            # PE transpose into the resident weight tile:
            # wqT[i, k, (t%4)*128+o] = wqv[o, k*128+i]
            dst = wqT_lo if t < 4 else wqT_hi
            off = (t % 4) * 128
            wps = ps_wtp.tile([128, GK, 128], BF16, name="wps")
            for k in range(GK):
                nc.tensor.transpose(
                    wps[:, k, :], wqv[:, k * 128:(k + 1) * 128], ident_b[:],
                )
            nc.scalar.copy(dst[:, :, off:off + 128], wps[:])

        def emit_xblock(b):
            if b + 5 < TB:
                xn_t = xin_pool.tile([128, DIN], F32, name="x_t")
                nc.sync.dma_start(
                    xn_t[:], x_ap[(b + 5) * 128:(b + 6) * 128, :],
                )
                x_pending.append(xn_t)
            x_t = x_pending.pop(0)   # tile for block b (issued 5 iters ago)
            nc.scalar.activation(
                junk[:], x_t[:], AF.Square, accum_out=ss_all[:, b:b + 1],
            )
            xb = xbf_pool.tile([128, DIN], BF16, name="xb")
            nc.vector.tensor_copy(xb[:], x_t[:])
            xps = ps_xtp.tile([128, GK, 128], BF16, name="xps")
            for k in range(GK):
                nc.tensor.transpose(
                    xps[:, k, :], xb[:, k * 128:(k + 1) * 128], ident_b[:],
                )
            if b % 2 == 0:
                nc.scalar.copy(xT[b][:], xps[:])
            else:
                nc.vector.tensor_copy(xT[b][:], xps[:])

        def emit_mm_h0(b):
            pm = ps_mm.tile([128, 512], F32, name="pm")
            for k in range(GK):
                nc.tensor.matmul(
                    pm[:], xT[b][:, k, :], wqT_lo[:, k, :],
                    start=(k == 0), stop=(k == GK - 1),
                )
            if b % 2 == 1:
                nc.scalar.copy(rh0[b][:], pm[:])
            else:
                nc.vector.tensor_copy(rh0[b][:], pm[:])

        def emit_mm_h1(b):
            pm = ps_mm.tile([128, 512], F32, name="pm")
            for k in range(GK):
                nc.tensor.matmul(
                    pm[:], xT[b][:, k, :], wqT_hi[:, k, :],
                    start=(k == 0), stop=(k == GK - 1),
                )
            ob = ob_pool.tile([128, DOUT], F32, name="ob")
            if b < STAGE_H1:
                nc.vector.tensor_copy(rh1[b][:], pm[:])
                nc.scalar.activation(
                    ob[:, 512:1024], rh1[b][:], AF.Copy,
                    scale=fac_all[:, b:b + 1],
                )
            else:
                nc.scalar.activation(
                    ob[:, 512:1024], pm[:], AF.Copy, scale=fac_all[:, b:b + 1],
                )
            nc.scalar.activation(
                ob[:, 0:512], rh0[b][:], AF.Copy, scale=fac_all[:, b:b + 1],
            )
            nc.gpsimd.dma_start(out_ap[b * 128:(b + 1) * 128, :], ob[:])

        # ---- emission ------------------------------------------------------
        x_pending = []
        for t in range(4):
            emit_wdma(t)
        for b in range(5):   # pre-issue x DMA lookahead
            x_t = xin_pool.tile([128, DIN], F32, name="x_t")
            nc.sync.dma_start(x_t[:], x_ap[b * 128:(b + 1) * 128, :])
            x_pending.append(x_t)
        for t in range(4):
            emit_wquant(t)

        w_insert = {6: 4, 12: 5, 18: 6, 24: 7}
        for b in range(TB):
            emit_xblock(b)
            if b in w_insert:
                t = w_insert[b]
                emit_wdma(t)
                emit_wquant(t)
            if b >= MM_LAG:
                emit_mm_h0(b - MM_LAG)
        for b in range(TB - MM_LAG, TB):
            emit_mm_h0(b)

        # ---- cross-core sum of squares (4 group-shards of a token half) ---
        nc.sync.dma_start(cc_in[:], ss_all[:])
        nc.gpsimd.collective_compute(
            "AllReduce",
            ALU.add,
            replica_groups=[[0, 1, 2, 3], [4, 5, 6, 7]],
            ins=[cc_in.opt()],
            outs=[cc_out.opt()],
        )
        nc.sync.dma_start(ss_sum[:], cc_out[:])
        nc.scalar.activation(
            sq_all[:], ss_sum[:], AF.Sqrt, bias=eps_t[:], scale=1.0 / DFULL,
        )
        nc.vector.reciprocal(fac_all[:], sq_all[:])

        # ---- pass 1 + finales ---------------------------------------------
        for b in range(TB):
            emit_mm_h1(b)


_NC_CACHE = None


def _ensure_ntff_hook():
    """Install the antenv.axon_hooks shim + ctypes NTFF hook if missing."""
    import types

    try:
        from antenv.axon_hooks import get_axon_ntff_profile_hook  # noqa: F401
        return
    except ImportError:
        pass
    import antenv

    mod = types.ModuleType("antenv.axon_hooks")
    mod._hook = None
    mod.set_axon_ntff_profile_hook = lambda h: setattr(mod, "_hook", h)
    mod.get_axon_ntff_profile_hook = lambda: mod._hook
    sys.modules["antenv.axon_hooks"] = mod
    antenv.axon_hooks = mod
    try:
        if "/root/.axon_site" not in sys.path:
            sys.path.insert(0, "/root/.axon_site")
        from trn_agent_boot.trn_boot import _ntff_profile_via_ctypes

        mod.set_axon_ntff_profile_hook(
            _ntff_profile_via_ctypes("/opt/axon/libaxon_pjrt.so")
        )
    except Exception:
        pass


def kernel(x: np.ndarray, weight: np.ndarray) -> np.ndarray:
    global LAST_EXEC_NS, LAST_RESULTS, _NC_CACHE
    x = np.ascontiguousarray(np.asarray(x, dtype=np.float32))
    weight = np.ascontiguousarray(np.asarray(weight, dtype=np.float32))
    lead = x.shape[:-1]
    xf = x.reshape(-1, DFULL)
    assert xf.shape[0] == 2 * TOK, xf.shape

    if _NC_CACHE is None:
        _NC_CACHE = _build()
    nc = _NC_CACHE

    in_maps = []
    for i in range(2):
        for j in range(4):
            in_maps.append({
                "x": np.ascontiguousarray(
                    xf[i * TOK:(i + 1) * TOK, j * DIN:(j + 1) * DIN]
                ),
                "weight": np.ascontiguousarray(
                    weight[j * DOUT:(j + 1) * DOUT, :]
                ),
            })
    trace = bool(int(os.environ.get("CCK_TRACE", "0")))
    kw = {}
    if trace:
        _ensure_ntff_hook()
        tdir = os.environ.get("CCK_TRACE_DIR")
        if tdir:
            os.makedirs(tdir, exist_ok=True)
            kw["tmpdir"] = tdir
    res = run_bass_kernel_spmd(nc, in_maps, list(range(N_CORES)), trace=trace, **kw)
    LAST_EXEC_NS = res.exec_time_ns
    LAST_RESULTS = res
    out = np.empty((2 * TOK, DFULL), dtype=np.float32)
    for i in range(2):
        for j in range(4):
            out[i * TOK:(i + 1) * TOK, j * DOUT:(j + 1) * DOUT] = (
                res.results[i * 4 + j]["out"]
            )
    return out.reshape(*lead, DFULL)


if __name__ == "__main__":
    rng = np.random.default_rng(0)
    x = rng.standard_normal((2, 4096, 4096), dtype=np.float32)
    w = (rng.standard_normal((4096, 1024), dtype=np.float32) * 0.02).astype(np.float32)
    o = kernel(x, w)
    print(o.shape, o.dtype, LAST_EXEC_NS)
